# revision 11
# baseline (speedup 1.0000x reference)
"""Additive (Bahdanau) attention Trainium2 kernel, 8-way batch-data-parallel.

Reference computation (per batch row b):
    enc_proj = enc_outputs @ W_enc.T                  # [S, A]
    dec_proj = dec_hidden @ W_dec.T                   # [A]
    score    = tanh(enc_proj + dec_proj) @ v[0]       # [S]
    attn     = softmax(score)                         # [S]
    context  = attn @ enc_outputs                     # [E]
Returns (context [B, E], attn_weights [B, S]).

B=64, S=2048, E=A=512.  Each of the 8 cores handles 8 batch rows.

Compute dtype: bf16 operands into the PE (inputs cast f32->bf16 for free by
the SWDGE cast-DMA), fp32 PSUM accumulation, fp32 softmax/normalization.

Per-core pipeline (per batch row b, per s-group g of 512 rows):
  - gpsimd cast-DMA enc[b, g] f32 HBM -> bf16 SBUF E_h [128p, 4j, 512e]
  - 16 PE transposes (bf16) -> PSUM -> DVE copies -> encT [e, s] bf16
  - 16 accumulating PE matmuls: proj[a-chunk, s512] = W_enc @ encT (f32 psum)
  - ACT tanh(proj + dec_projT[a, b]) -> SBUF bf16
  - 4 PE matmuls: score[1, s512] = vT.T @ tanh (f32 psum)
  - ACT exp(score) -> w_row slice (f32), accum_out -> per-group denom partial
  - 4 PE transposes of w (f32) -> wT [s128, 1] chunks -> DVE copy -> bf16
  - 4 PE matmuls accumulate context[1, 512] += wT.T @ E_h (f32 psum)
  After 4 groups: denom = sum of partials; attn = w/denom; context /= denom.

No max-subtraction in the softmax: |score| <= ||v||_1 ~= 11.3, so exp() is
safe in f32 and matches jax.nn.softmax to fp rounding.
"""

import numpy as np

import concourse.bass as bass
import concourse.mybir as mybir
import concourse.tile as tile
from concourse.bass_utils import run_bass_kernel_spmd
from concourse.masks import make_identity
from concourse.vector_clock import ScopedClock

# ---------------------------------------------------------------------------
# Workarounds: this walrus build rejects any instruction carrying more than
# one semaphore wait.  Split extra waits onto same-engine nops, and do the
# same for the TileContext tail drain.
# ---------------------------------------------------------------------------
_wsplit_counter = [0]


def _split_multi_waits(nc):
    for fn in nc.m.functions:
        for bb in fn.blocks:
            insts = bb.instructions
            if not any(
                i.sync_info is not None
                and i.sync_info.on_wait
                and len(i.sync_info.on_wait) > 1
                for i in insts
            ):
                continue
            new = []
            for inst in insts:
                si = inst.sync_info
                if si is not None and si.on_wait and len(si.on_wait) > 1:
                    waits = list(si.on_wait)
                    for w in waits[:-1]:
                        nop = mybir.InstNoOp(name=f"I-wsplit-{_wsplit_counter[0]}")
                        _wsplit_counter[0] += 1
                        nop.engine = inst.engine
                        nop.bass_nofuse = True
                        nop.sync_info = mybir.SyncInfo(on_wait=[w], on_update=[])
                        new.append(nop)
                    si.on_wait = waits[-1:]
                new.append(inst)
            bb.instructions = new


def _patched_drain_and_barrier(self, tick_clock, wait_clock):
    nc = self.nc
    probe = nc.sync.nop(nofuse=True)
    wait_clock.add_sem_waits(probe.ins, ScopedClock({None: tick_clock.global_clock}))
    si = probe.ins.sync_info
    waits = list(si.on_wait) if si and si.on_wait else []
    if len(waits) > 1:
        si.on_wait = waits[:1]
        for w in waits[1:]:
            extra = nc.sync.nop(nofuse=True)
            esi = extra.ins.sync_info
            if esi is None:
                extra.ins.sync_info = mybir.SyncInfo(on_wait=[w], on_update=[])
            else:
                esi.on_wait = [w]
    nc.sync.drain()
    nc.all_engine_barrier()
    popped = nc._tile_sem_poison_stack.pop()
    assert popped is self._sem_poison
    nc.clear_and_free_semaphores(list(self.sems.allocated().values()))
    nc.all_engine_barrier()


tile.TileContext._drain_and_barrier = _patched_drain_and_barrier

# ---------------------------------------------------------------------------
# Problem shapes (hardcoded per spec)
# ---------------------------------------------------------------------------
B, S, E, A = 64, 2048, 512, 512
N_CORES = 8
BL = B // N_CORES          # batch rows per core
SG = 512                   # s-rows per group
NG = S // SG               # groups per batch row (4)
NSUB = SG // 128           # 128-row subtiles per group (4)
NCH = E // 128             # 128-wide chunks of E/A (4)

F32 = mybir.dt.float32
BF16 = mybir.dt.bfloat16
AF = mybir.ActivationFunctionType


def build_kernel():
    nc = bass.Bass("TRN2", target_bir_lowering=False, debug=False)

    enc = nc.declare_dram_parameter("enc_outputs", [BL, S, E], F32, isOutput=False)
    dec = nc.declare_dram_parameter("dec_hidden", [BL, E], F32, isOutput=False)
    w_enc = nc.declare_dram_parameter("W_enc", [A, E], F32, isOutput=False)
    w_dec = nc.declare_dram_parameter("W_dec", [A, E], F32, isOutput=False)
    v = nc.declare_dram_parameter("v", [1, A], F32, isOutput=False)
    ctx_out = nc.declare_dram_parameter("context", [BL, E], F32, isOutput=True)
    attn_out = nc.declare_dram_parameter("attn_weights", [BL, S], F32, isOutput=True)

    with tile.TileContext(nc) as tc:
        const_cm = tc.tile_pool(name="const", bufs=1)
        const = const_cm.__enter__()

        ident = const.tile([128, 128], F32)
        make_identity(nc, ident)
        ident_h = const.tile([128, 128], BF16)
        nc.vector.tensor_copy(ident_h, ident)

        # prefetch batch-row-0 enc tiles before any setup so the SWDGE
        # cast-DMAs start immediately (they gate the whole PE pipeline)
        p_enc_cm = tc.tile_pool(name="enc_h", bufs=4)
        p_enc = p_enc_cm.__enter__()
        prefetched = {}
        for g in range(NG):
            tiles = []
            for j in range(NSUB):
                t = p_enc.tile([128, E], BF16, tag=f"e_h{j}")
                nc.gpsimd.dma_start(
                    out=t,
                    in_=enc.ap()[0, g * SG + j * 128:g * SG + (j + 1) * 128, :],
                )
                tiles.append(t)
            prefetched[g] = tiles

        # ---- setup: transpose W_enc -> WencT blocks [e128, (ec,ac), a128] --
        with tc.tile_pool(name="setup_sb", bufs=1) as ssb, \
             tc.tile_pool(name="setup_ps", bufs=2, space="PSUM") as sps:
            wenc_nat = ssb.tile([128, NCH, E], F32)  # [a128, ac, e]
            nc.sync.dma_start(
                out=wenc_nat,
                in_=w_enc.ap().rearrange("(ac p) e -> p ac e", p=128),
            )
            wencT = const.tile([128, NCH * NCH, 128], BF16)  # [(ec*4+ac)]
            for ec in range(NCH):
                for ac in range(NCH):
                    pt = sps.tile([128, 128], F32, tag="wt_ps")
                    nc.tensor.transpose(
                        pt, wenc_nat[:, ac, ec * 128:(ec + 1) * 128], ident
                    )
                    nc.vector.tensor_copy(wencT[:, ec * NCH + ac, :], pt)

            # ---- W_dec transposed blocks (plain f32; only feeds f32 mm) ----
            wdec_nat = ssb.tile([128, NCH, E], F32)  # [a128, ac, d]
            nc.sync.dma_start(
                out=wdec_nat,
                in_=w_dec.ap().rearrange("(ac p) e -> p ac e", p=128),
            )
            wdecT = ssb.tile([128, NCH * NCH, 128], BF16)
            for dc in range(NCH):
                for ac in range(NCH):
                    pt = sps.tile([128, 128], F32, tag="dech_ps")
                    nc.tensor.transpose(
                        pt, wdec_nat[:, ac, dc * 128:(dc + 1) * 128], ident
                    )
                    nc.vector.tensor_copy(wdecT[:, dc * NCH + ac, :], pt)

            # ---- dec_hidden transposed: [d128, dc, b] --------------------
            dec_sb = ssb.tile([BL, E], F32)
            nc.sync.dma_start(out=dec_sb, in_=dec.ap())
            dechT = ssb.tile([128, NCH, BL], BF16)
            for dc in range(NCH):
                pt = sps.tile([128, BL], F32, tag="dech_ps")
                nc.tensor.transpose(
                    pt, dec_sb[:, dc * 128:(dc + 1) * 128], ident[0:BL, 0:BL]
                )
                nc.vector.tensor_copy(dechT[:, dc, :], pt)

            # ---- dec_projT[a, b] = sum_d W_dec[a, d] dec_hidden[b, d] ----
            dp_ps = sps.tile([128, NCH, BL], F32, tag="dp_ps")
            for ac in range(NCH):
                for dc in range(NCH):
                    nc.tensor.matmul(
                        dp_ps[:, ac, :],
                        wdecT[:, dc * NCH + ac, :],
                        dechT[:, dc, :],
                        start=(dc == 0),
                        stop=(dc == NCH - 1),
                    )
            dec_projT = const.tile([128, NCH, BL], F32)
            nc.vector.tensor_copy(dec_projT, dp_ps)

            # ---- v transposed: [a128, ac] --------------------------------
            v_sb = ssb.tile([1, A], F32)
            nc.sync.dma_start(out=v_sb, in_=v.ap())
            vT_ps = sps.tile([128, NCH], F32, tag="vt_ps")
            for ac in range(NCH):
                nc.tensor.transpose(
                    vT_ps[:, ac:ac + 1],
                    v_sb[0:1, ac * 128:(ac + 1) * 128],
                    ident[0:1, 0:1],
                )
            vT = const.tile([128, NCH], BF16)
            nc.vector.tensor_copy(vT, vT_ps)

        # ------------------- main pools -----------------------------------
        with tc.tile_pool(name="encT", bufs=3) as p_encT, \
             tc.tile_pool(name="tanh", bufs=3) as p_tanh, \
             tc.tile_pool(name="small", bufs=2) as p_small, \
             tc.tile_pool(name="ps_tr", bufs=2, space="PSUM") as ps_tr, \
             tc.tile_pool(name="ps_proj", bufs=2, space="PSUM") as ps_proj, \
             tc.tile_pool(name="ps_score", bufs=1, space="PSUM") as ps_score, \
             tc.tile_pool(name="ps_w", bufs=1, space="PSUM") as ps_w, \
             tc.tile_pool(name="ps_ctx", bufs=2, space="PSUM") as ps_ctx:

            for b in range(BL):
                w_row = p_small.tile([1, S], F32, tag="w_row")
                den_parts = p_small.tile([1, NG], F32, tag="den_parts")
                ctx_ps = ps_ctx.tile([1, E], F32, tag="ctx")

                for g in range(NG):
                    # cast-load enc rows [g*512, (g+1)*512) as bf16 [p, j, e]
                    if b == 0:
                        e_h = prefetched[g]
                    else:
                        e_h = []
                        for j in range(NSUB):
                            t = p_enc.tile([128, E], BF16, tag=f"e_h{j}")
                            nc.gpsimd.dma_start(
                                out=t,
                                in_=enc.ap()[b, g * SG + j * 128:
                                             g * SG + (j + 1) * 128, :],
                            )
                            e_h.append(t)

                    # transpose to encT [e128, ec, s512] (bf16)
                    encT = p_encT.tile([128, NCH, SG], BF16, tag="encT")
                    for ec in range(NCH):
                        pt = ps_tr.tile([128, SG], BF16, tag="tr")
                        for j in range(NSUB):
                            nc.tensor.transpose(
                                pt[:, j * 128:(j + 1) * 128],
                                e_h[j][:, ec * 128:(ec + 1) * 128],
                                ident_h,
                            )
                        nc.vector.tensor_copy(encT[:, ec, :], pt)

                    # proj + tanh + score
                    score_ps = ps_score.tile([1, SG], F32, tag="score")
                    for ac in range(NCH):
                        proj = ps_proj.tile([128, SG], F32, tag="proj")
                        for ec in range(NCH):
                            nc.tensor.matmul(
                                proj,
                                wencT[:, ec * NCH + ac, :],
                                encT[:, ec, :],
                                start=(ec == 0),
                                stop=(ec == NCH - 1),
                            )
                        th = p_tanh.tile([128, SG], BF16, tag="tanh")
                        nc.scalar.activation(
                            th, proj, AF.Tanh,
                            bias=dec_projT[:, ac, b:b + 1], scale=1.0,
                        )
                        nc.tensor.matmul(
                            score_ps,
                            vT[:, ac:ac + 1],
                            th,
                            start=(ac == 0),
                            stop=(ac == NCH - 1),
                        )

                    # exp (no max subtraction; scores are small by construction)
                    nc.scalar.activation(
                        w_row[0:1, g * SG:(g + 1) * SG], score_ps, AF.Exp,
                        accum_out=den_parts[0:1, g:g + 1],
                    )

                    # transpose w to [s128, t] chunks (f32 in, bf16 out via copy)
                    wT_ps = ps_w.tile([128, NSUB], F32, tag="wT")
                    for t in range(NSUB):
                        nc.tensor.transpose(
                            wT_ps[:, t:t + 1],
                            w_row[0:1, g * SG + t * 128:g * SG + (t + 1) * 128],
                            ident[0:1, 0:1],
                        )
                    wT = p_small.tile([128, NSUB], BF16, tag="wT_sb")
                    nc.vector.tensor_copy(wT, wT_ps)

                    # context += wT.T @ e_h   (accumulates across groups)
                    for t in range(NSUB):
                        nc.tensor.matmul(
                            ctx_ps,
                            wT[:, t:t + 1],
                            e_h[t],
                            start=(g == 0 and t == 0),
                            stop=(g == NG - 1 and t == NSUB - 1),
                        )

                # epilogue for batch row b
                den = p_small.tile([1, 1], F32, tag="den")
                nc.vector.tensor_reduce(
                    den, den_parts, axis=mybir.AxisListType.X, op=mybir.AluOpType.add
                )
                rden = p_small.tile([1, 1], F32, tag="rden")
                nc.vector.reciprocal(rden, den)

                attn_row = p_small.tile([1, S], F32, tag="attn_row")
                nc.vector.tensor_scalar_mul(attn_row, w_row, rden)
                nc.sync.dma_start(out=attn_out.ap()[b:b + 1, :], in_=attn_row)

                ctx_row = p_small.tile([1, E], F32, tag="ctx_row")
                nc.vector.tensor_scalar_mul(ctx_row, ctx_ps, rden)
                nc.sync.dma_start(out=ctx_out.ap()[b:b + 1, :], in_=ctx_row)

        p_enc_cm.__exit__(None, None, None)
        const_cm.__exit__(None, None, None)

    _split_multi_waits(nc)
    return nc


_NC_CACHE = None


def _get_nc():
    global _NC_CACHE
    if _NC_CACHE is None:
        _NC_CACHE = build_kernel()
    return _NC_CACHE


def kernel(**inputs):
    enc = np.ascontiguousarray(np.asarray(inputs["enc_outputs"], dtype=np.float32))
    dec = np.ascontiguousarray(np.asarray(inputs["dec_hidden"], dtype=np.float32))
    w_enc = np.ascontiguousarray(np.asarray(inputs["W_enc"], dtype=np.float32))
    w_dec = np.ascontiguousarray(np.asarray(inputs["W_dec"], dtype=np.float32))
    v = np.ascontiguousarray(np.asarray(inputs["v"], dtype=np.float32))

    nc = _get_nc()
    in_maps = []
    for c in range(N_CORES):
        sl = slice(c * BL, (c + 1) * BL)
        in_maps.append({
            "enc_outputs": enc[sl],
            "dec_hidden": dec[sl],
            "W_enc": w_enc,
            "W_dec": w_dec,
            "v": v,
        })
    res = run_bass_kernel_spmd(nc, in_maps, core_ids=list(range(N_CORES)))
    context = np.concatenate([res.results[c]["context"] for c in range(N_CORES)], axis=0)
    attn = np.concatenate([res.results[c]["attn_weights"] for c in range(N_CORES)], axis=0)
    return (context, attn)


# revision 12
# speedup vs baseline: 1.0080x; 1.0080x over previous
"""Additive (Bahdanau) attention Trainium2 kernel, 8-way batch-data-parallel.

Reference computation (per batch row b):
    enc_proj = enc_outputs @ W_enc.T                  # [S, A]
    dec_proj = dec_hidden @ W_dec.T                   # [A]
    score    = tanh(enc_proj + dec_proj) @ v[0]       # [S]
    attn     = softmax(score)                         # [S]
    context  = attn @ enc_outputs                     # [E]
Returns (context [B, E], attn_weights [B, S]).

B=64, S=2048, E=A=512.  Each of the 8 cores handles 8 batch rows.

Compute dtype: bf16 operands into the PE (inputs cast f32->bf16 for free by
the SWDGE cast-DMA), fp32 PSUM accumulation, fp32 softmax/normalization.

Per-core pipeline (per batch row b, per s-group g of 512 rows):
  - gpsimd cast-DMA enc[b, g] f32 HBM -> bf16 SBUF E_h [128p, 4j, 512e]
  - 16 PE transposes (bf16) -> PSUM -> DVE copies -> encT [e, s] bf16
  - 16 accumulating PE matmuls: proj[a-chunk, s512] = W_enc @ encT (f32 psum)
  - ACT tanh(proj + dec_projT[a, b]) -> SBUF bf16
  - 4 PE matmuls: score[1, s512] = vT.T @ tanh (f32 psum)
  - ACT exp(score) -> w_row slice (f32), accum_out -> per-group denom partial
  - 4 PE transposes of w (f32) -> wT [s128, 1] chunks -> DVE copy -> bf16
  - 4 PE matmuls accumulate context[1, 512] += wT.T @ E_h (f32 psum)
  After 4 groups: denom = sum of partials; attn = w/denom; context /= denom.

No max-subtraction in the softmax: |score| <= ||v||_1 ~= 11.3, so exp() is
safe in f32 and matches jax.nn.softmax to fp rounding.
"""

import numpy as np

import concourse.bass as bass
import concourse.mybir as mybir
import concourse.tile as tile
from concourse.bass_utils import run_bass_kernel_spmd
from concourse.masks import make_identity
from concourse.vector_clock import ScopedClock

# ---------------------------------------------------------------------------
# Workarounds: this walrus build rejects any instruction carrying more than
# one semaphore wait.  Split extra waits onto same-engine nops, and do the
# same for the TileContext tail drain.
# ---------------------------------------------------------------------------
_wsplit_counter = [0]


def _split_multi_waits(nc):
    for fn in nc.m.functions:
        for bb in fn.blocks:
            insts = bb.instructions
            if not any(
                i.sync_info is not None
                and i.sync_info.on_wait
                and len(i.sync_info.on_wait) > 1
                for i in insts
            ):
                continue
            new = []
            for inst in insts:
                si = inst.sync_info
                if si is not None and si.on_wait and len(si.on_wait) > 1:
                    waits = list(si.on_wait)
                    for w in waits[:-1]:
                        nop = mybir.InstNoOp(name=f"I-wsplit-{_wsplit_counter[0]}")
                        _wsplit_counter[0] += 1
                        nop.engine = inst.engine
                        nop.bass_nofuse = True
                        nop.sync_info = mybir.SyncInfo(on_wait=[w], on_update=[])
                        new.append(nop)
                    si.on_wait = waits[-1:]
                new.append(inst)
            bb.instructions = new


def _patched_drain_and_barrier(self, tick_clock, wait_clock):
    nc = self.nc
    probe = nc.sync.nop(nofuse=True)
    wait_clock.add_sem_waits(probe.ins, ScopedClock({None: tick_clock.global_clock}))
    si = probe.ins.sync_info
    waits = list(si.on_wait) if si and si.on_wait else []
    if len(waits) > 1:
        si.on_wait = waits[:1]
        for w in waits[1:]:
            extra = nc.sync.nop(nofuse=True)
            esi = extra.ins.sync_info
            if esi is None:
                extra.ins.sync_info = mybir.SyncInfo(on_wait=[w], on_update=[])
            else:
                esi.on_wait = [w]
    nc.sync.drain()
    nc.all_engine_barrier()
    popped = nc._tile_sem_poison_stack.pop()
    assert popped is self._sem_poison
    nc.clear_and_free_semaphores(list(self.sems.allocated().values()))
    nc.all_engine_barrier()


tile.TileContext._drain_and_barrier = _patched_drain_and_barrier

# ---------------------------------------------------------------------------
# Problem shapes (hardcoded per spec)
# ---------------------------------------------------------------------------
B, S, E, A = 64, 2048, 512, 512
N_CORES = 8
BL = B // N_CORES          # batch rows per core
SG = 512                   # s-rows per group
NG = S // SG               # groups per batch row (4)
NSUB = SG // 128           # 128-row subtiles per group (4)
NCH = E // 128             # 128-wide chunks of E/A (4)

F32 = mybir.dt.float32
BF16 = mybir.dt.bfloat16
AF = mybir.ActivationFunctionType


def build_kernel():
    nc = bass.Bass("TRN2", target_bir_lowering=False, debug=False)

    enc = nc.declare_dram_parameter("enc_outputs", [BL, S, E], F32, isOutput=False)
    dec = nc.declare_dram_parameter("dec_hidden", [BL, E], F32, isOutput=False)
    w_enc = nc.declare_dram_parameter("W_enc", [A, E], F32, isOutput=False)
    w_dec = nc.declare_dram_parameter("W_dec", [A, E], F32, isOutput=False)
    v = nc.declare_dram_parameter("v", [1, A], F32, isOutput=False)
    ctx_out = nc.declare_dram_parameter("context", [BL, E], F32, isOutput=True)
    attn_out = nc.declare_dram_parameter("attn_weights", [BL, S], F32, isOutput=True)

    with tile.TileContext(nc) as tc:
        const_cm = tc.tile_pool(name="const", bufs=1)
        const = const_cm.__enter__()

        ident = const.tile([128, 128], F32)
        make_identity(nc, ident)
        ident_h = const.tile([128, 128], BF16)
        nc.vector.tensor_copy(ident_h, ident)

        # prefetch batch-row-0 enc tiles before any setup so the SWDGE
        # cast-DMAs start immediately (they gate the whole PE pipeline)
        p_enc_cm = tc.tile_pool(name="enc_h", bufs=4)
        p_enc = p_enc_cm.__enter__()
        prefetched = {}
        for g in range(NG):
            e_h = p_enc.tile([128, NSUB, E], BF16, tag="e_h")
            nc.gpsimd.dma_start(
                out=e_h,
                in_=enc.ap()[0, g * SG:(g + 1) * SG, :].rearrange(
                    "(j p) e -> p j e", p=128
                ),
            )
            prefetched[g] = e_h

        # ---- setup: transpose W_enc -> WencT blocks [e128, (ec,ac), a128] --
        with tc.tile_pool(name="setup_sb", bufs=1) as ssb, \
             tc.tile_pool(name="setup_ps", bufs=2, space="PSUM") as sps:
            wenc_nat = ssb.tile([128, NCH, E], F32)  # [a128, ac, e]
            nc.sync.dma_start(
                out=wenc_nat,
                in_=w_enc.ap().rearrange("(ac p) e -> p ac e", p=128),
            )
            wencT = const.tile([128, NCH * NCH, 128], BF16)  # [(ec*4+ac)]
            for ec in range(NCH):
                for ac in range(NCH):
                    pt = sps.tile([128, 128], F32, tag="wt_ps")
                    nc.tensor.transpose(
                        pt, wenc_nat[:, ac, ec * 128:(ec + 1) * 128], ident
                    )
                    nc.vector.tensor_copy(wencT[:, ec * NCH + ac, :], pt)

            # ---- W_dec transposed blocks (plain f32; only feeds f32 mm) ----
            wdec_nat = ssb.tile([128, NCH, E], F32)  # [a128, ac, d]
            nc.sync.dma_start(
                out=wdec_nat,
                in_=w_dec.ap().rearrange("(ac p) e -> p ac e", p=128),
            )
            wdecT = ssb.tile([128, NCH * NCH, 128], BF16)
            for dc in range(NCH):
                for ac in range(NCH):
                    pt = sps.tile([128, 128], F32, tag="dech_ps")
                    nc.tensor.transpose(
                        pt, wdec_nat[:, ac, dc * 128:(dc + 1) * 128], ident
                    )
                    nc.vector.tensor_copy(wdecT[:, dc * NCH + ac, :], pt)

            # ---- dec_hidden transposed: [d128, dc, b] --------------------
            dec_sb = ssb.tile([BL, E], F32)
            nc.sync.dma_start(out=dec_sb, in_=dec.ap())
            dechT = ssb.tile([128, NCH, BL], BF16)
            for dc in range(NCH):
                pt = sps.tile([128, BL], F32, tag="dech_ps")
                nc.tensor.transpose(
                    pt, dec_sb[:, dc * 128:(dc + 1) * 128], ident[0:BL, 0:BL]
                )
                nc.vector.tensor_copy(dechT[:, dc, :], pt)

            # ---- dec_projT[a, b] = sum_d W_dec[a, d] dec_hidden[b, d] ----
            dp_ps = sps.tile([128, NCH, BL], F32, tag="dp_ps")
            for ac in range(NCH):
                for dc in range(NCH):
                    nc.tensor.matmul(
                        dp_ps[:, ac, :],
                        wdecT[:, dc * NCH + ac, :],
                        dechT[:, dc, :],
                        start=(dc == 0),
                        stop=(dc == NCH - 1),
                    )
            dec_projT = const.tile([128, NCH, BL], F32)
            nc.vector.tensor_copy(dec_projT, dp_ps)

            # ---- v transposed: [a128, ac] --------------------------------
            v_sb = ssb.tile([1, A], F32)
            nc.sync.dma_start(out=v_sb, in_=v.ap())
            vT_ps = sps.tile([128, NCH], F32, tag="vt_ps")
            for ac in range(NCH):
                nc.tensor.transpose(
                    vT_ps[:, ac:ac + 1],
                    v_sb[0:1, ac * 128:(ac + 1) * 128],
                    ident[0:1, 0:1],
                )
            vT = const.tile([128, NCH], BF16)
            nc.vector.tensor_copy(vT, vT_ps)

        # ------------------- main pools -----------------------------------
        with tc.tile_pool(name="encT", bufs=4) as p_encT, \
             tc.tile_pool(name="tanh", bufs=4) as p_tanh, \
             tc.tile_pool(name="small", bufs=2) as p_small, \
             tc.tile_pool(name="ps_tr", bufs=2, space="PSUM") as ps_tr, \
             tc.tile_pool(name="ps_proj", bufs=2, space="PSUM") as ps_proj, \
             tc.tile_pool(name="ps_score", bufs=1, space="PSUM") as ps_score, \
             tc.tile_pool(name="ps_w", bufs=1, space="PSUM") as ps_w, \
             tc.tile_pool(name="ps_ctx", bufs=2, space="PSUM") as ps_ctx:

            for b in range(BL):
                w_row = p_small.tile([1, S], F32, tag="w_row")
                den_parts = p_small.tile([1, NG], F32, tag="den_parts")
                ctx_ps = ps_ctx.tile([1, E], F32, tag="ctx")

                for g in range(NG):
                    # cast-load enc rows [g*512, (g+1)*512) as bf16 [p, j, e]
                    if b == 0:
                        e_h = prefetched[g]
                    else:
                        e_h = p_enc.tile([128, NSUB, E], BF16, tag="e_h")
                        nc.gpsimd.dma_start(
                            out=e_h,
                            in_=enc.ap()[b, g * SG:(g + 1) * SG, :].rearrange(
                                "(j p) e -> p j e", p=128
                            ),
                        )

                    # transpose to encT [e128, ec, s512] (bf16)
                    encT = p_encT.tile([128, NCH, SG], BF16, tag="encT")
                    for ec in range(NCH):
                        pt = ps_tr.tile([128, SG], BF16, tag="tr")
                        for j in range(NSUB):
                            nc.tensor.transpose(
                                pt[:, j * 128:(j + 1) * 128],
                                e_h[:, j, ec * 128:(ec + 1) * 128],
                                ident_h,
                            )
                        nc.vector.tensor_copy(encT[:, ec, :], pt)

                    # proj + tanh + score
                    score_ps = ps_score.tile([1, SG], F32, tag="score")
                    for ac in range(NCH):
                        proj = ps_proj.tile([128, SG], F32, tag="proj")
                        for ec in range(NCH):
                            nc.tensor.matmul(
                                proj,
                                wencT[:, ec * NCH + ac, :],
                                encT[:, ec, :],
                                start=(ec == 0),
                                stop=(ec == NCH - 1),
                            )
                        th = p_tanh.tile([128, SG], BF16, tag="tanh")
                        nc.scalar.activation(
                            th, proj, AF.Tanh,
                            bias=dec_projT[:, ac, b:b + 1], scale=1.0,
                        )
                        nc.tensor.matmul(
                            score_ps,
                            vT[:, ac:ac + 1],
                            th,
                            start=(ac == 0),
                            stop=(ac == NCH - 1),
                        )

                    # exp (no max subtraction; scores are small by construction)
                    nc.scalar.activation(
                        w_row[0:1, g * SG:(g + 1) * SG], score_ps, AF.Exp,
                        accum_out=den_parts[0:1, g:g + 1],
                    )

                    # transpose w to [s128, t] chunks (f32 in, bf16 out via copy)
                    wT_ps = ps_w.tile([128, NSUB], F32, tag="wT")
                    for t in range(NSUB):
                        nc.tensor.transpose(
                            wT_ps[:, t:t + 1],
                            w_row[0:1, g * SG + t * 128:g * SG + (t + 1) * 128],
                            ident[0:1, 0:1],
                        )
                    wT = p_small.tile([128, NSUB], BF16, tag="wT_sb")
                    nc.vector.tensor_copy(wT, wT_ps)

                    # context += wT.T @ e_h   (accumulates across groups)
                    for t in range(NSUB):
                        nc.tensor.matmul(
                            ctx_ps,
                            wT[:, t:t + 1],
                            e_h[:, t, :],
                            start=(g == 0 and t == 0),
                            stop=(g == NG - 1 and t == NSUB - 1),
                        )

                # epilogue for batch row b
                den = p_small.tile([1, 1], F32, tag="den")
                nc.vector.tensor_reduce(
                    den, den_parts, axis=mybir.AxisListType.X, op=mybir.AluOpType.add
                )
                rden = p_small.tile([1, 1], F32, tag="rden")
                nc.vector.reciprocal(rden, den)

                attn_row = p_small.tile([1, S], F32, tag="attn_row")
                nc.vector.tensor_scalar_mul(attn_row, w_row, rden)
                nc.sync.dma_start(out=attn_out.ap()[b:b + 1, :], in_=attn_row)

                ctx_row = p_small.tile([1, E], F32, tag="ctx_row")
                nc.vector.tensor_scalar_mul(ctx_row, ctx_ps, rden)
                nc.sync.dma_start(out=ctx_out.ap()[b:b + 1, :], in_=ctx_row)

        p_enc_cm.__exit__(None, None, None)
        const_cm.__exit__(None, None, None)

    _split_multi_waits(nc)
    return nc


_NC_CACHE = None


def _get_nc():
    global _NC_CACHE
    if _NC_CACHE is None:
        _NC_CACHE = build_kernel()
    return _NC_CACHE


def kernel(**inputs):
    enc = np.ascontiguousarray(np.asarray(inputs["enc_outputs"], dtype=np.float32))
    dec = np.ascontiguousarray(np.asarray(inputs["dec_hidden"], dtype=np.float32))
    w_enc = np.ascontiguousarray(np.asarray(inputs["W_enc"], dtype=np.float32))
    w_dec = np.ascontiguousarray(np.asarray(inputs["W_dec"], dtype=np.float32))
    v = np.ascontiguousarray(np.asarray(inputs["v"], dtype=np.float32))

    nc = _get_nc()
    in_maps = []
    for c in range(N_CORES):
        sl = slice(c * BL, (c + 1) * BL)
        in_maps.append({
            "enc_outputs": enc[sl],
            "dec_hidden": dec[sl],
            "W_enc": w_enc,
            "W_dec": w_dec,
            "v": v,
        })
    res = run_bass_kernel_spmd(nc, in_maps, core_ids=list(range(N_CORES)))
    context = np.concatenate([res.results[c]["context"] for c in range(N_CORES)], axis=0)
    attn = np.concatenate([res.results[c]["attn_weights"] for c in range(N_CORES)], axis=0)
    return (context, attn)


# revision 13
# speedup vs baseline: 1.0595x; 1.0511x over previous
"""Additive (Bahdanau) attention Trainium2 kernel, 8-way batch-data-parallel.

Reference computation (per batch row b):
    enc_proj = enc_outputs @ W_enc.T                  # [S, A]
    dec_proj = dec_hidden @ W_dec.T                   # [A]
    score    = tanh(enc_proj + dec_proj) @ v[0]       # [S]
    attn     = softmax(score)                         # [S]
    context  = attn @ enc_outputs                     # [E]
Returns (context [B, E], attn_weights [B, S]).

B=64, S=2048, E=A=512.  Each of the 8 cores handles 8 batch rows.

Compute dtype: bf16 operands into the PE (inputs cast f32->bf16 for free by
the SWDGE cast-DMA), fp32 PSUM accumulation, fp32 softmax/normalization.

Per-core pipeline (per batch row b, per s-group g of 512 rows):
  - gpsimd cast-DMA enc[b, g] f32 HBM -> bf16 SBUF E_h [128p, 4j, 512e]
  - 16 PE transposes (bf16) -> PSUM -> DVE copies -> encT [e, s] bf16
  - 16 accumulating PE matmuls: proj[a-chunk, s512] = W_enc @ encT (f32 psum)
  - ACT tanh(proj + dec_projT[a, b]) -> SBUF bf16
  - 4 PE matmuls: score[1, s512] = vT.T @ tanh (f32 psum)
  - ACT exp(score) -> w_row slice (f32), accum_out -> per-group denom partial
  - 4 PE transposes of w (f32) -> wT [s128, 1] chunks -> DVE copy -> bf16
  - 4 PE matmuls accumulate context[1, 512] += wT.T @ E_h (f32 psum)
  After 4 groups: denom = sum of partials; attn = w/denom; context /= denom.

No max-subtraction in the softmax: |score| <= ||v||_1 ~= 11.3, so exp() is
safe in f32 and matches jax.nn.softmax to fp rounding.
"""

import numpy as np

import concourse.bass as bass
import concourse.mybir as mybir
import concourse.tile as tile
from concourse.bass_utils import run_bass_kernel_spmd
from concourse.masks import make_identity
from concourse.vector_clock import ScopedClock

# ---------------------------------------------------------------------------
# Workarounds: this walrus build rejects any instruction carrying more than
# one semaphore wait.  Split extra waits onto same-engine nops, and do the
# same for the TileContext tail drain.
# ---------------------------------------------------------------------------
_wsplit_counter = [0]


def _split_multi_waits(nc):
    for fn in nc.m.functions:
        for bb in fn.blocks:
            insts = bb.instructions
            if not any(
                i.sync_info is not None
                and i.sync_info.on_wait
                and len(i.sync_info.on_wait) > 1
                for i in insts
            ):
                continue
            new = []
            for inst in insts:
                si = inst.sync_info
                if si is not None and si.on_wait and len(si.on_wait) > 1:
                    waits = list(si.on_wait)
                    for w in waits[:-1]:
                        nop = mybir.InstNoOp(name=f"I-wsplit-{_wsplit_counter[0]}")
                        _wsplit_counter[0] += 1
                        nop.engine = inst.engine
                        nop.bass_nofuse = True
                        nop.sync_info = mybir.SyncInfo(on_wait=[w], on_update=[])
                        new.append(nop)
                    si.on_wait = waits[-1:]
                new.append(inst)
            bb.instructions = new


def _patched_drain_and_barrier(self, tick_clock, wait_clock):
    nc = self.nc
    probe = nc.sync.nop(nofuse=True)
    wait_clock.add_sem_waits(probe.ins, ScopedClock({None: tick_clock.global_clock}))
    si = probe.ins.sync_info
    waits = list(si.on_wait) if si and si.on_wait else []
    if len(waits) > 1:
        si.on_wait = waits[:1]
        for w in waits[1:]:
            extra = nc.sync.nop(nofuse=True)
            esi = extra.ins.sync_info
            if esi is None:
                extra.ins.sync_info = mybir.SyncInfo(on_wait=[w], on_update=[])
            else:
                esi.on_wait = [w]
    nc.sync.drain()
    nc.all_engine_barrier()
    popped = nc._tile_sem_poison_stack.pop()
    assert popped is self._sem_poison
    nc.clear_and_free_semaphores(list(self.sems.allocated().values()))
    nc.all_engine_barrier()


tile.TileContext._drain_and_barrier = _patched_drain_and_barrier

# ---------------------------------------------------------------------------
# Problem shapes (hardcoded per spec)
# ---------------------------------------------------------------------------
B, S, E, A = 64, 2048, 512, 512
N_CORES = 8
BL = B // N_CORES          # batch rows per core
SG = 512                   # s-rows per group
NG = S // SG               # groups per batch row (4)
NSUB = SG // 128           # 128-row subtiles per group (4)
NCH = E // 128             # 128-wide chunks of E/A (4)

F32 = mybir.dt.float32
BF16 = mybir.dt.bfloat16
AF = mybir.ActivationFunctionType


def build_kernel():
    nc = bass.Bass("TRN2", target_bir_lowering=False, debug=False)

    enc = nc.declare_dram_parameter("enc_outputs", [BL, S, E], F32, isOutput=False)
    dec = nc.declare_dram_parameter("dec_hidden", [BL, E], F32, isOutput=False)
    w_enc = nc.declare_dram_parameter("W_enc", [A, E], F32, isOutput=False)
    w_dec = nc.declare_dram_parameter("W_dec", [A, E], F32, isOutput=False)
    v = nc.declare_dram_parameter("v", [1, A], F32, isOutput=False)
    ctx_out = nc.declare_dram_parameter("context", [BL, E], F32, isOutput=True)
    attn_out = nc.declare_dram_parameter("attn_weights", [BL, S], F32, isOutput=True)

    with tile.TileContext(nc) as tc:
        const_cm = tc.tile_pool(name="const", bufs=1)
        const = const_cm.__enter__()

        ident = const.tile([128, 128], F32)
        make_identity(nc, ident)
        ident_h = const.tile([128, 128], BF16)
        nc.vector.tensor_copy(ident_h, ident)

        # prefetch batch-row-0 enc tiles before any setup so the SWDGE
        # cast-DMAs start immediately (they gate the whole PE pipeline)
        p_enc_cm = tc.tile_pool(name="enc_h", bufs=4)
        p_enc = p_enc_cm.__enter__()
        prefetched = {}
        for g in range(NG):
            e_h = p_enc.tile([128, NSUB, E], BF16, tag="e_h")
            nc.gpsimd.dma_start(
                out=e_h,
                in_=enc.ap()[0, g * SG:(g + 1) * SG, :].rearrange(
                    "(j p) e -> p j e", p=128
                ),
            )
            prefetched[g] = e_h

        # ---- setup: transpose W_enc -> WencT blocks [e128, (ec,ac), a128] --
        with tc.tile_pool(name="setup_sb", bufs=1) as ssb, \
             tc.tile_pool(name="setup_ps", bufs=2, space="PSUM") as sps:
            wenc_nat = ssb.tile([128, NCH, E], F32)  # [a128, ac, e]
            nc.sync.dma_start(
                out=wenc_nat,
                in_=w_enc.ap().rearrange("(ac p) e -> p ac e", p=128),
            )
            wencT = const.tile([128, NCH * NCH, 128], BF16)  # [(ec*4+ac)]
            for ec in range(NCH):
                for ac in range(NCH):
                    pt = sps.tile([128, 128], F32, tag="wt_ps")
                    nc.tensor.transpose(
                        pt, wenc_nat[:, ac, ec * 128:(ec + 1) * 128], ident
                    )
                    nc.vector.tensor_copy(wencT[:, ec * NCH + ac, :], pt)

            # ---- W_dec transposed blocks (plain f32; only feeds f32 mm) ----
            wdec_nat = ssb.tile([128, NCH, E], F32)  # [a128, ac, d]
            nc.sync.dma_start(
                out=wdec_nat,
                in_=w_dec.ap().rearrange("(ac p) e -> p ac e", p=128),
            )
            wdecT = ssb.tile([128, NCH * NCH, 128], BF16)
            for dc in range(NCH):
                for ac in range(NCH):
                    pt = sps.tile([128, 128], F32, tag="dech_ps")
                    nc.tensor.transpose(
                        pt, wdec_nat[:, ac, dc * 128:(dc + 1) * 128], ident
                    )
                    nc.vector.tensor_copy(wdecT[:, dc * NCH + ac, :], pt)

            # ---- dec_hidden transposed: [d128, dc, b] --------------------
            dec_sb = ssb.tile([BL, E], F32)
            nc.sync.dma_start(out=dec_sb, in_=dec.ap())
            dechT = ssb.tile([128, NCH, BL], BF16)
            for dc in range(NCH):
                pt = sps.tile([128, BL], F32, tag="dech_ps")
                nc.tensor.transpose(
                    pt, dec_sb[:, dc * 128:(dc + 1) * 128], ident[0:BL, 0:BL]
                )
                nc.vector.tensor_copy(dechT[:, dc, :], pt)

            # ---- dec_projT[a, b] = sum_d W_dec[a, d] dec_hidden[b, d] ----
            dp_ps = sps.tile([128, NCH, BL], F32, tag="dp_ps")
            for ac in range(NCH):
                for dc in range(NCH):
                    nc.tensor.matmul(
                        dp_ps[:, ac, :],
                        wdecT[:, dc * NCH + ac, :],
                        dechT[:, dc, :],
                        start=(dc == 0),
                        stop=(dc == NCH - 1),
                    )
            dec_projT = const.tile([128, NCH, BL], F32)
            nc.vector.tensor_copy(dec_projT, dp_ps)

            # ---- v transposed: [a128, ac] --------------------------------
            v_sb = ssb.tile([1, A], F32)
            nc.sync.dma_start(out=v_sb, in_=v.ap())
            vT_ps = sps.tile([128, NCH], F32, tag="vt_ps")
            for ac in range(NCH):
                nc.tensor.transpose(
                    vT_ps[:, ac:ac + 1],
                    v_sb[0:1, ac * 128:(ac + 1) * 128],
                    ident[0:1, 0:1],
                )
            vT = const.tile([128, NCH], BF16)
            nc.vector.tensor_copy(vT, vT_ps)

        # ------------------- main pools -----------------------------------
        with tc.tile_pool(name="encT", bufs=4) as p_encT, \
             tc.tile_pool(name="tanh", bufs=4) as p_tanh, \
             tc.tile_pool(name="small", bufs=2) as p_small, \
             tc.tile_pool(name="ps_tr", bufs=2, space="PSUM") as ps_tr, \
             tc.tile_pool(name="ps_proj", bufs=2, space="PSUM") as ps_proj, \
             tc.tile_pool(name="ps_score", bufs=1, space="PSUM") as ps_score, \
             tc.tile_pool(name="ps_w", bufs=1, space="PSUM") as ps_w, \
             tc.tile_pool(name="ps_ctx", bufs=2, space="PSUM") as ps_ctx:

            for b in range(BL):
                w_row = p_small.tile([1, S], F32, tag="w_row")
                den_parts = p_small.tile([1, NG], F32, tag="den_parts")
                ctx_ps = ps_ctx.tile([1, E], F32, tag="ctx")

                for g in range(NG):
                    # cast-load enc rows [g*512, (g+1)*512) as bf16 [p, j, e]
                    if b == 0:
                        e_h = prefetched[g]
                    else:
                        e_h = p_enc.tile([128, NSUB, E], BF16, tag="e_h")
                        nc.gpsimd.dma_start(
                            out=e_h,
                            in_=enc.ap()[b, g * SG:(g + 1) * SG, :].rearrange(
                                "(j p) e -> p j e", p=128
                            ),
                        )

                    # transpose to encT [e128, ec, s512] (bf16); stage two
                    # e-chunks per PSUM bank so proj only waits twice per group
                    encT = p_encT.tile([128, NCH, SG], BF16, tag="encT")
                    for eh in range(NCH // 2):
                        pt = ps_tr.tile([128, 2, SG], BF16, tag="tr")
                        for ei in range(2):
                            ec = eh * 2 + ei
                            for j in range(NSUB):
                                nc.tensor.transpose(
                                    pt[:, ei, j * 128:(j + 1) * 128],
                                    e_h[:, j, ec * 128:(ec + 1) * 128],
                                    ident_h,
                                )
                        nc.vector.tensor_copy(encT[:, eh * 2:eh * 2 + 2, :], pt)

                    # proj + tanh + score
                    score_ps = ps_score.tile([1, SG], F32, tag="score")
                    for ac in range(NCH):
                        proj = ps_proj.tile([128, SG], F32, tag="proj")
                        for ec in range(NCH):
                            nc.tensor.matmul(
                                proj,
                                wencT[:, ec * NCH + ac, :],
                                encT[:, ec, :],
                                start=(ec == 0),
                                stop=(ec == NCH - 1),
                            )
                        th = p_tanh.tile([128, SG], BF16, tag="tanh")
                        nc.scalar.activation(
                            th, proj, AF.Tanh,
                            bias=dec_projT[:, ac, b:b + 1], scale=1.0,
                        )
                        nc.tensor.matmul(
                            score_ps,
                            vT[:, ac:ac + 1],
                            th,
                            start=(ac == 0),
                            stop=(ac == NCH - 1),
                        )

                    # exp (no max subtraction; scores are small by construction)
                    nc.scalar.activation(
                        w_row[0:1, g * SG:(g + 1) * SG], score_ps, AF.Exp,
                        accum_out=den_parts[0:1, g:g + 1],
                    )

                    # transpose w to [s128, t] chunks (f32 in, bf16 out via copy)
                    wT_ps = ps_w.tile([128, NSUB], F32, tag="wT")
                    for t in range(NSUB):
                        nc.tensor.transpose(
                            wT_ps[:, t:t + 1],
                            w_row[0:1, g * SG + t * 128:g * SG + (t + 1) * 128],
                            ident[0:1, 0:1],
                        )
                    wT = p_small.tile([128, NSUB], BF16, tag="wT_sb")
                    nc.vector.tensor_copy(wT, wT_ps)

                    # context += wT.T @ e_h   (accumulates across groups)
                    for t in range(NSUB):
                        nc.tensor.matmul(
                            ctx_ps,
                            wT[:, t:t + 1],
                            e_h[:, t, :],
                            start=(g == 0 and t == 0),
                            stop=(g == NG - 1 and t == NSUB - 1),
                        )

                # epilogue for batch row b
                den = p_small.tile([1, 1], F32, tag="den")
                nc.vector.tensor_reduce(
                    den, den_parts, axis=mybir.AxisListType.X, op=mybir.AluOpType.add
                )
                rden = p_small.tile([1, 1], F32, tag="rden")
                nc.vector.reciprocal(rden, den)

                attn_row = p_small.tile([1, S], F32, tag="attn_row")
                nc.vector.tensor_scalar_mul(attn_row, w_row, rden)
                nc.sync.dma_start(out=attn_out.ap()[b:b + 1, :], in_=attn_row)

                ctx_row = p_small.tile([1, E], F32, tag="ctx_row")
                nc.vector.tensor_scalar_mul(ctx_row, ctx_ps, rden)
                nc.sync.dma_start(out=ctx_out.ap()[b:b + 1, :], in_=ctx_row)

        p_enc_cm.__exit__(None, None, None)
        const_cm.__exit__(None, None, None)

    _split_multi_waits(nc)
    return nc


_NC_CACHE = None


def _get_nc():
    global _NC_CACHE
    if _NC_CACHE is None:
        _NC_CACHE = build_kernel()
    return _NC_CACHE


def kernel(**inputs):
    enc = np.ascontiguousarray(np.asarray(inputs["enc_outputs"], dtype=np.float32))
    dec = np.ascontiguousarray(np.asarray(inputs["dec_hidden"], dtype=np.float32))
    w_enc = np.ascontiguousarray(np.asarray(inputs["W_enc"], dtype=np.float32))
    w_dec = np.ascontiguousarray(np.asarray(inputs["W_dec"], dtype=np.float32))
    v = np.ascontiguousarray(np.asarray(inputs["v"], dtype=np.float32))

    nc = _get_nc()
    in_maps = []
    for c in range(N_CORES):
        sl = slice(c * BL, (c + 1) * BL)
        in_maps.append({
            "enc_outputs": enc[sl],
            "dec_hidden": dec[sl],
            "W_enc": w_enc,
            "W_dec": w_dec,
            "v": v,
        })
    res = run_bass_kernel_spmd(nc, in_maps, core_ids=list(range(N_CORES)))
    context = np.concatenate([res.results[c]["context"] for c in range(N_CORES)], axis=0)
    attn = np.concatenate([res.results[c]["attn_weights"] for c in range(N_CORES)], axis=0)
    return (context, attn)
